# revision 12
# baseline (speedup 1.0000x reference)
"""Bass/Tile TRN2 kernel: 16-head MHA (B=2, T=2048, D=1024, H=64) on 8 NeuronCores.

Sharding: 8-way tensor parallel over heads — core c computes heads {2c, 2c+1}
for BOTH batches. Output sharding is (batch, 256-row tq slice): after each
batch's attention, one AllToAll swaps head-shards for tq-slice shards and the
core runs the output projection for its 256-row slice of that batch. The b=0
AllToAll + output projection overlap b=1's attention; only b=1's 512KB
collective is exposed at the tail.

Per-core pipeline (everything bf16 into the PE, fp32 PSUM accumulation):
  - Inputs arrive pre-transposed ([D, T]) and are DMA'd in 512-column chunks
    so the first V-projection matmul issues ~4us in.
  - QKV projections: 8x [128,128]x[128,512] accumulating matmuls per block.
  - Scores S^T[tk, tq] = K^T.T @ Q^T per head, written to PSUM as bf16 so a
    [128, 2048] score tile (4 tk-chunks of one head) is 2 banks; 1/sqrt(H) is
    folded into Wq/bq on host.
  - exp on ScalarE straight out of PSUM in 2048-wide ACTIVATEs (ACT is the
    critical engine: ~(N+352)/1.2ns, dtype-independent). Per-head score tiles
    ping-pong so ACT never waits on score matmuls.
  - PV matmul with a ones-augmented V (stationary col 64 = ones) so row 64 of
    the PV accumulator is the softmax denominator for free.
  - Normalize: DVE reciprocal of the denominator row, GPSIMD
    partition-broadcast to 64 rows, DVE multiply (psum read) -> staged shard.
  - PE idle slots inside the ACT-bound attention phase are filled with the
    next batch's projections and the previous batch's output projection
    (software pipelining keeps the PE HAM-warm at 2.4 GHz).
Host does layout-only prep (transpose, bf16 cast, weight slicing) and
concatenates the 8 cores' 2x256-row output slices.
"""

import sys
from contextlib import ExitStack

import numpy as np

sys.path.insert(0, "/opt/trn_rl_repo")

import ml_dtypes  # noqa: E402

BF16 = ml_dtypes.bfloat16

B, T, D = 2, 2048, 1024
N_HEADS, H = 16, 64
NCORES = 8
GROUPS = [[0, 1, 2, 3, 4, 5, 6, 7]]
NLOC = 2            # heads per core
TQB = 512           # attention tq block
NTQB = T // TQB     # 4
TKC = 128           # tk chunk
NTKC = T // TKC     # 16
RG = 2              # tk chunks per exp group
NG = NTKC // RG     # 8 groups per block
DC = 128            # d chunk
NDC = D // DC       # 8
XB = 512            # x-load column block
SLICE = 256         # output tq rows per core per batch
VA = 128            # V_aug stationary width: [V(64) | ones(1) | junk(63)]
NW = NLOC * H       # 128 projection width per core

_CACHE = {}


def _legalize_waits(bir_bytes):
    """This toolchain's walrus accepts at most ONE semaphore wait per
    instruction ("Too many sync wait commands"). Tile's sem assignment emits
    several. Hoist all but one wait of each instruction onto same-engine NoOps
    inserted immediately before it (engines execute their stream in order, so
    waiting earlier on the same engine is equivalent)."""
    import json

    j = json.loads(bir_bytes)
    ctr = 0
    for fn in j["functions"]:
        for blk in fn["blocks"]:
            out = []
            for ins in blk["instructions"]:
                si = ins.get("sync_info")
                waits = (si or {}).get("on_wait") or []
                if len(waits) > 1:
                    for w in waits[:-1]:
                        ctr += 1
                        out.append(
                            {
                                "engine": ins["engine"],
                                "ins": [],
                                "outs": [],
                                "name": f"waitfix-{ctr}",
                                "opcode": "NoOp",
                                "sync_info": {"on_wait": [w], "on_update": []},
                            }
                        )
                    si["on_wait"] = [waits[-1]]
                out.append(ins)
            blk["instructions"] = out
    return json.dumps(j).encode()


def _build():
    import concourse.bass as bass
    import concourse.mybir as mybir
    import concourse.tile as tile

    f32 = mybir.dt.float32
    bf16 = mybir.dt.bfloat16
    AF = mybir.ActivationFunctionType
    ALU = mybir.AluOpType

    nc = bass.Bass(
        "TRN2", target_bir_lowering=False, debug=False, num_devices=NCORES
    )

    qT = [nc.dram_tensor(f"qT{b}", [D, T], bf16, kind="ExternalInput") for b in range(B)]
    kT = [nc.dram_tensor(f"kT{b}", [D, T], bf16, kind="ExternalInput") for b in range(B)]
    vT = [nc.dram_tensor(f"vT{b}", [D, T], bf16, kind="ExternalInput") for b in range(B)]
    wq = nc.dram_tensor("wq", [D, NW], bf16, kind="ExternalInput")
    wk = nc.dram_tensor("wk", [D, NW], bf16, kind="ExternalInput")
    wv = nc.dram_tensor("wv", [D, NW], bf16, kind="ExternalInput")
    wp = nc.dram_tensor("wp", [N_HEADS * H, D], bf16, kind="ExternalInput")
    bq = nc.dram_tensor("bq", [128, 1], f32, kind="ExternalInput")
    bk = nc.dram_tensor("bk", [128, 1], f32, kind="ExternalInput")
    bv = nc.dram_tensor("bv", [128, 1], f32, kind="ExternalInput")
    bp = nc.dram_tensor("bp", [128, D], f32, kind="ExternalInput")
    ident = nc.dram_tensor("ident", [128, 128], bf16, kind="ExternalInput")
    out = nc.dram_tensor("out", [B * SLICE, D], f32, kind="ExternalOutput")

    with tile.TileContext(nc) as tc, ExitStack() as ctx:
        p_const = ctx.enter_context(tc.tile_pool(name="const", bufs=1))
        p_x = ctx.enter_context(tc.tile_pool(name="x", bufs=1))
        p_qk = ctx.enter_context(tc.tile_pool(name="qk", bufs=2))
        p_va = ctx.enter_context(tc.tile_pool(name="va", bufs=2))
        p_pt = ctx.enter_context(tc.tile_pool(name="pt", bufs=2))
        p_a = ctx.enter_context(tc.tile_pool(name="a", bufs=2))
        p_o = ctx.enter_context(tc.tile_pool(name="o", bufs=2))
        p_dram = ctx.enter_context(tc.tile_pool(name="dram", bufs=1, space="DRAM"))

        ps_ss = ctx.enter_context(tc.tile_pool(name="ps_ss", bufs=1, space="PSUM"))
        ps_pv = ctx.enter_context(tc.tile_pool(name="ps_pv", bufs=1, space="PSUM"))
        ps_mm = ctx.enter_context(tc.tile_pool(name="ps_mm", bufs=2, space="PSUM"))

        # ---- constant loads -------------------------------------------------
        wq_sb = p_const.tile([128, NDC * NW], bf16)
        wk_sb = p_const.tile([128, NDC * NW], bf16)
        wv_sb = p_const.tile([128, NDC * NW], bf16)
        wp_sb = p_const.tile([128, (N_HEADS * H // 128) * D], bf16)
        bq_sb = p_const.tile([128, 1], f32)
        bk_sb = p_const.tile([128, 1], f32)
        bv_sb = p_const.tile([128, 1], f32)
        bp_sb = p_const.tile([128, D], f32)
        id_sb = p_const.tile([128, 128], bf16)
        nc.sync.dma_start(id_sb[:], ident[:])
        for sb_t, ext, m in (
            (wq_sb, wq, NW),
            (wk_sb, wk, NW),
            (wv_sb, wv, NW),
            (wp_sb, wp, D),
        ):
            nc.sync.dma_start(
                sb_t[:].rearrange("p (c m) -> p c m", m=m),
                ext[:].rearrange("(c p) m -> p c m", p=128),
            )
        for sb_t, ext in ((bq_sb, bq), (bk_sb, bk), (bv_sb, bv), (bp_sb, bp)):
            nc.sync.dma_start(sb_t[:], ext[:])

        # warm the ACT exp table while everything else is still loading
        warm = p_const.tile([1, 8], bf16)
        nc.vector.memset(warm[:], 0.0)
        nc.scalar.activation(warm[:], warm[:], AF.Exp)

        # ones column (f32, to match the f32 reciprocal row) for the
        # reciprocal-row broadcast outer product
        ones_sb = p_const.tile([128, H], f32)
        nc.vector.memset(ones_sb[:], 1.0)

        a2a_in = [p_dram.tile([NCORES * NW, SLICE], bf16, name=f"a2ai{b}") for b in range(B)]
        a2a_out = [p_dram.tile([NCORES * NW, SLICE], bf16, name=f"a2ao{b}") for b in range(B)]

        # ---- x loads: per (tensor, tb) 512-col chunks -----------------------
        def load_x(b):
            xs = {}
            for tname, ext in (("v", vT[b]), ("k", kT[b]), ("q", qT[b])):
                for tb in range(NTQB):
                    t_ = p_x.tile(
                        [128, NDC * XB], bf16,
                        name=f"x{tname}{b}{tb}", tag=f"x{tname}{tb}",
                    )
                    nc.sync.dma_start(
                        t_[:].rearrange("p (c t) -> p c t", t=XB),
                        ext[:, tb * XB : (tb + 1) * XB].rearrange(
                            "(c p) t -> p c t", p=128
                        ),
                    )
                    xs[(tname, tb)] = t_
            return xs

        # ---- projection pieces (emitted inline or as PE fillers) ------------
        def proj_block(w_sb, b_sb, xt, dst, dcol, b, tag):
            ps = ps_mm.tile([128, XB], f32, name=f"mm{tag}{b}{dcol}", tag="mm")
            for dc in range(NDC):
                nc.tensor.matmul(
                    ps[:],
                    lhsT=w_sb[:, dc * NW : (dc + 1) * NW],
                    rhs=xt[:, dc * XB : (dc + 1) * XB],
                    start=(dc == 0),
                    stop=(dc == NDC - 1),
                )
            nc.vector.tensor_scalar(
                dst[:, dcol * XB : (dcol + 1) * XB], ps[:], b_sb[:, 0:1], None,
                ALU.add,
            )

        def va_piece(b, vt, va, i0, n):
            """Transpose tk-chunks [i0, i0+n) of vt into va."""
            for i in range(i0, i0 + n):
                pst = ps_mm.tile([128, 128], bf16, name=f"pst{b}{i}", tag="mm")
                nc.tensor.transpose(pst[:], vt[:, i * TKC : (i + 1) * TKC], id_sb[:])
                dst = va[:, i * NLOC * VA : (i + 1) * NLOC * VA].rearrange(
                    "p (h x) -> p h x", x=VA
                )[:, :, 0:H]
                nc.vector.tensor_copy(dst, pst[:].rearrange("p (h x) -> p h x", x=H))

        def make_proj(b, xs):
            """Returns (qt, kt, va, fillers): fillers produce vt/va/kt and
            qt blocks 1..3; caller must run the first pieces inline for b=0."""
            vt = p_va.tile([128, T], bf16, name=f"vt{b}", tag="vt")
            va = p_va.tile([128, NTKC * NLOC * VA], bf16, name=f"va{b}", tag="va")
            qt = p_qk.tile([128, T], bf16, name=f"qt{b}", tag="qt")
            kt = p_qk.tile([128, T], bf16, name=f"kt{b}", tag="kt")
            nc.vector.memset(
                va[:].rearrange("p (i h x) -> p i h x", h=NLOC, x=VA)[
                    :, :, :, H : H + 1
                ],
                1.0,
            )
            pieces = []
            for tb in range(NTQB):
                pieces.append(
                    lambda tb=tb: proj_block(wv_sb, bv_sb, xs[("v", tb)], vt, tb, b, "v")
                )
            for i0 in range(0, NTKC, 4):
                pieces.append(lambda i0=i0: va_piece(b, vt, va, i0, 4))
            for tb in range(NTQB):
                pieces.append(
                    lambda tb=tb: proj_block(wk_sb, bk_sb, xs[("k", tb)], kt, tb, b, "k")
                )
            for tb in range(NTQB):
                pieces.append(
                    lambda tb=tb: proj_block(wq_sb, bq_sb, xs[("q", tb)], qt, tb, b, "q")
                )
            return qt, kt, va, pieces

        def outproj_pieces(b):
            """Output projection for this core's 256-row slice of batch b."""
            ats = p_a.tile([128, NCORES * SLICE], bf16, name=f"ats{b}", tag="ats")

            def load():
                nc.sync.dma_start(
                    ats[:].rearrange("p (c t) -> p c t", t=SLICE),
                    a2a_out[b][:].rearrange("(c p) t -> p c t", p=128),
                )

            pieces = [load]
            for tqc in range(SLICE // 128):
                for dh in range(2):
                    def op(tqc=tqc, dh=dh):
                        ps = ps_mm.tile(
                            [128, 512], f32, name=f"po{b}{tqc}{dh}", tag="mm"
                        )
                        for nhc in range(NCORES):
                            nc.tensor.matmul(
                                ps[:],
                                lhsT=ats[:, nhc * SLICE + tqc * 128 : nhc * SLICE + (tqc + 1) * 128],
                                rhs=wp_sb[:, nhc * D + dh * 512 : nhc * D + (dh + 1) * 512],
                                start=(nhc == 0),
                                stop=(nhc == NCORES - 1),
                            )
                        o_sb = p_o.tile(
                            [128, 512], f32, name=f"o{b}{tqc}{dh}", tag="o"
                        )
                        nc.vector.tensor_tensor(
                            o_sb[:], ps[:], bp_sb[:, dh * 512 : (dh + 1) * 512],
                            ALU.add,
                        )
                        nc.sync.dma_start(
                            out[
                                b * SLICE + tqc * 128 : b * SLICE + (tqc + 1) * 128,
                                dh * 512 : (dh + 1) * 512,
                            ],
                            o_sb[:],
                        )
                    pieces.append(op)
            return pieces

        fillers = []

        def run_filler(n=1):
            for _ in range(n):
                if fillers:
                    fillers.pop(0)()

        def attn_block(b, j, qt, kt, va, fill_groups=None):
            """One (batch, tq-block) attention block, software-pipelined:
            scores(g+1) are emitted before PV(g) so the PE keeps busy while
            ACT exps group g; filler pieces run after selected groups."""
            pv = [
                ps_pv.tile([VA, TQB], f32, name=f"pv{b}{j}{hd}", tag=f"pv{hd}")
                for hd in range(NLOC)
            ]
            pss = {}
            pt = {}

            def scores(g):
                for hd in range(NLOC):
                    pss[(g % 2, hd)] = ps_ss.tile(
                        [128, RG * TQB], f32,
                        name=f"pss{b}{j}{g}{hd}", tag=f"ss{hd}",
                    )
                for r in range(g * RG, (g + 1) * RG):
                    for hd in range(NLOC):
                        nc.tensor.matmul(
                            pss[(g % 2, hd)][:, (r % RG) * TQB : (r % RG + 1) * TQB],
                            lhsT=kt[hd * H : (hd + 1) * H, r * TKC : (r + 1) * TKC],
                            rhs=qt[hd * H : (hd + 1) * H, j * TQB : (j + 1) * TQB],
                            start=True,
                            stop=True,
                        )

            def exps(g):
                for hd in range(NLOC):
                    pt[(g % 2, hd)] = p_pt.tile(
                        [128, RG * TQB], bf16,
                        name=f"pt{b}{j}{g}{hd}", tag=f"pt{hd}",
                    )
                    nc.scalar.activation(
                        pt[(g % 2, hd)][:], pss[(g % 2, hd)][:], AF.Exp
                    )

            def pvs(g):
                for hd in range(NLOC):
                    for r in range(g * RG, (g + 1) * RG):
                        col0 = (r * NLOC + hd) * VA
                        nc.tensor.matmul(
                            pv[hd][:],
                            lhsT=va[:, col0 : col0 + VA],
                            rhs=pt[(g % 2, hd)][:, (r % RG) * TQB : (r % RG + 1) * TQB],
                            start=(r == g * RG and g == 0),
                            stop=(r == (g + 1) * RG - 1 and g == NG - 1),
                        )

            scores(0)
            exps(0)
            for g in range(NG):
                if g + 1 < NG:
                    scores(g + 1)
                    exps(g + 1)
                pvs(g)
                if fill_groups is None or g in fill_groups:
                    run_filler()

            # normalize per head (all DVE operands at matching partition
            # offsets: denom sits at psum partition 64, so the reciprocal
            # lands at partition 64 of rec_t; a K=1 PE outer product against
            # a ones column broadcasts it back to partitions 0..63).
            ans = []
            for hd in range(NLOC):
                rec_t = p_a.tile([H + 1, TQB], f32, name=f"rec{b}{j}{hd}", tag="rec")
                nc.vector.reciprocal(rec_t[H : H + 1, :], pv[hd][H : H + 1, :])
                rep_ps = ps_mm.tile([H, TQB], f32, name=f"repp{b}{j}{hd}", tag="mm")
                nc.tensor.matmul(
                    rep_ps[:],
                    lhsT=ones_sb[H : H + 1, 0:H],
                    rhs=rec_t[H : H + 1, :],
                    start=True,
                    stop=True,
                )
                rep = p_a.tile([H, TQB], f32, name=f"rep{b}{j}{hd}", tag="rep")
                nc.vector.tensor_copy(rep[:], rep_ps[:])
                an = p_a.tile([H, TQB], bf16, name=f"an{b}{j}{hd}", tag=f"an{hd}")
                nc.vector.tensor_tensor(an[:], pv[hd][0:H, :], rep[:], ALU.mult)
                ans.append(an)
            for half in range(2):
                s = 2 * j + half
                for hd in range(NLOC):
                    nc.sync.dma_start(
                        a2a_in[b][s * NW + hd * H : s * NW + (hd + 1) * H, :],
                        ans[hd][:, half * SLICE : (half + 1) * SLICE],
                    )

        # ===== main schedule =================================================
        xs0 = load_x(0)
        qt0, kt0, va0, pieces0 = make_proj(0, xs0)
        # inline: V, va, K, Q(tb=0); leave Q(tb 1..3) as fillers
        for p in pieces0[:13]:
            p()
        fillers.extend(pieces0[13:])

        xs1 = load_x(1)
        qt1, kt1, va1, pieces1 = make_proj(1, xs1)
        fillers.extend(pieces1)

        for j in range(NTQB):
            attn_block(0, j, qt0, kt0, va0)
        nc.gpsimd.collective_compute(
            "AllToAll",
            ALU.bypass,
            replica_groups=GROUPS,
            ins=[a2a_in[0].opt()],
            outs=[a2a_out[0].opt()],
        )
        fillers.extend(outproj_pieces(0))

        # pop fillers only on groups 3/7 during b=1: the b=0 output
        # projection pieces then land ~35us+ after the b=0 AllToAll was
        # issued, so the PE does not stall waiting for it.
        for j in range(NTQB):
            attn_block(1, j, qt1, kt1, va1, fill_groups=(3, 7))
        nc.gpsimd.collective_compute(
            "AllToAll",
            ALU.bypass,
            replica_groups=GROUPS,
            ins=[a2a_in[1].opt()],
            outs=[a2a_out[1].opt()],
        )
        run_filler(len(fillers))
        for p in outproj_pieces(1):
            p()

    orig_to_json = nc.to_json_bytes
    nc.to_json_bytes = lambda: _legalize_waits(orig_to_json())
    return nc


def _get_nc():
    if "nc" not in _CACHE:
        _CACHE["nc"] = _build()
    return _CACHE["nc"]


def _make_in_maps(inputs):
    q = np.asarray(inputs["q"], dtype=np.float32)
    v = np.asarray(inputs["v"], dtype=np.float32)
    k = np.asarray(inputs["k"], dtype=np.float32)
    w_query = np.asarray(inputs["w_query"], dtype=np.float32)
    b_query = np.asarray(inputs["b_query"], dtype=np.float32)
    w_value = np.asarray(inputs["w_value"], dtype=np.float32)
    b_value = np.asarray(inputs["b_value"], dtype=np.float32)
    w_key = np.asarray(inputs["w_key"], dtype=np.float32)
    b_key = np.asarray(inputs["b_key"], dtype=np.float32)
    w_projection = np.asarray(inputs["w_projection"], dtype=np.float32)
    b_projection = np.asarray(inputs["b_projection"], dtype=np.float32)

    scale = np.float32(1.0 / np.sqrt(H))
    wp_s = np.ascontiguousarray(
        w_projection.transpose(0, 2, 1).reshape(N_HEADS * H, D)
    ).astype(BF16)
    bp_s = np.ascontiguousarray(
        np.tile(b_projection.reshape(1, D), (128, 1))
    ).astype(np.float32)

    xT = {}
    for b in range(B):
        xT[b] = tuple(
            np.ascontiguousarray(x[b].T).astype(BF16) for x in (q, k, v)
        )

    in_maps = []
    for c in range(NCORES):
        hs = c * NLOC
        wq_s = (w_query[:, hs : hs + NLOC, :].reshape(D, NW) * scale).astype(BF16)
        wk_s = w_key[:, hs : hs + NLOC, :].reshape(D, NW).astype(BF16)
        wv_s = w_value[:, hs : hs + NLOC, :].reshape(D, NW).astype(BF16)
        bq_s = np.ascontiguousarray(
            (b_query[hs : hs + NLOC].reshape(NW) * scale).reshape(NW, 1)
        ).astype(np.float32)
        bk_s = np.ascontiguousarray(
            b_key[hs : hs + NLOC].reshape(NW, 1)
        ).astype(np.float32)
        bv_s = np.ascontiguousarray(
            b_value[hs : hs + NLOC].reshape(NW, 1)
        ).astype(np.float32)
        m = {
            "ident": np.eye(128, dtype=np.float32).astype(BF16),
            "wq": np.ascontiguousarray(wq_s),
            "wk": np.ascontiguousarray(wk_s),
            "wv": np.ascontiguousarray(wv_s),
            "wp": wp_s,
            "bq": bq_s,
            "bk": bk_s,
            "bv": bv_s,
            "bp": bp_s,
        }
        for b in range(B):
            m[f"qT{b}"], m[f"kT{b}"], m[f"vT{b}"] = xT[b]
        in_maps.append(m)
    return in_maps


def _assemble(results):
    out = np.empty((B, T, D), np.float32)
    for c in range(NCORES):
        res = results[c]["out"]
        for b in range(B):
            out[b, c * SLICE : (c + 1) * SLICE, :] = res[
                b * SLICE : (b + 1) * SLICE
            ]
    return out


def run(inputs, trace=False, **kwargs):
    from concourse.bass_utils import run_bass_kernel_spmd

    nc = _get_nc()
    in_maps = _make_in_maps(inputs)
    res = run_bass_kernel_spmd(
        nc, in_maps, list(range(NCORES)), trace=trace, **kwargs
    )
    return _assemble(res.results), res


def kernel(**inputs) -> np.ndarray:
    out, _ = run(inputs, trace=False)
    return out


# revision 22
# speedup vs baseline: 1.0062x; 1.0062x over previous
"""Bass/Tile TRN2 kernel: 16-head MHA (B=2, T=2048, D=1024, H=64) on 8 NeuronCores.

Sharding: 8-way tensor parallel over heads — core c computes heads {2c, 2c+1}
for BOTH batches. Output sharding is (batch, 256-row tq slice): after each
batch's attention, one AllToAll swaps head-shards for tq-slice shards and the
core runs the output projection for its 256-row slice of that batch. The b=0
AllToAll + output projection overlap b=1's attention; only b=1's 512KB
collective is exposed at the tail.

Per-core pipeline (everything bf16 into the PE, fp32 PSUM accumulation):
  - Inputs arrive pre-transposed ([D, T]) and are DMA'd in 512-column chunks
    so the first V-projection matmul issues ~4us in.
  - QKV projections: 8x [128,128]x[128,512] accumulating matmuls per block.
  - Scores S^T[tk, tq] = K^T.T @ Q^T per head, written to PSUM as bf16 so a
    [128, 2048] score tile (4 tk-chunks of one head) is 2 banks; 1/sqrt(H) is
    folded into Wq/bq on host.
  - exp on ScalarE straight out of PSUM in 2048-wide ACTIVATEs (ACT is the
    critical engine: ~(N+352)/1.2ns, dtype-independent). Per-head score tiles
    ping-pong so ACT never waits on score matmuls.
  - PV matmul with a ones-augmented V (stationary col 64 = ones) so row 64 of
    the PV accumulator is the softmax denominator for free.
  - Normalize: DVE reciprocal of the denominator row, GPSIMD
    partition-broadcast to 64 rows, DVE multiply (psum read) -> staged shard.
  - PE idle slots inside the ACT-bound attention phase are filled with the
    next batch's projections and the previous batch's output projection
    (software pipelining keeps the PE HAM-warm at 2.4 GHz).
Host does layout-only prep (transpose, bf16 cast, weight slicing) and
concatenates the 8 cores' 2x256-row output slices.
"""

import sys
from contextlib import ExitStack

import numpy as np

sys.path.insert(0, "/opt/trn_rl_repo")

import ml_dtypes  # noqa: E402

BF16 = ml_dtypes.bfloat16

B, T, D = 2, 2048, 1024
N_HEADS, H = 16, 64
NCORES = 8
GROUPS = [[0, 1, 2, 3, 4, 5, 6, 7]]
NLOC = 2            # heads per core
TQB = 512           # attention tq block
NTQB = T // TQB     # 4
TKC = 128           # tk chunk
NTKC = T // TKC     # 16
RG = 2              # tk chunks per exp group
NG = NTKC // RG     # 8 groups per block
DC = 128            # d chunk
NDC = D // DC       # 8
XB = 512            # x-load column block
SLICE = 256         # output tq rows per core per batch
VA = 128            # V_aug stationary width: [V(64) | ones(1) | junk(63)]
NW = NLOC * H       # 128 projection width per core

_CACHE = {}


def _legalize_waits(bir_bytes):
    """This toolchain's walrus accepts at most ONE semaphore wait per
    instruction ("Too many sync wait commands"). Tile's sem assignment emits
    several. Hoist all but one wait of each instruction onto same-engine NoOps
    inserted immediately before it (engines execute their stream in order, so
    waiting earlier on the same engine is equivalent)."""
    import json

    j = json.loads(bir_bytes)
    ctr = 0
    for fn in j["functions"]:
        for blk in fn["blocks"]:
            out = []
            for ins in blk["instructions"]:
                si = ins.get("sync_info")
                waits = (si or {}).get("on_wait") or []
                if len(waits) > 1:
                    for w in waits[:-1]:
                        ctr += 1
                        out.append(
                            {
                                "engine": ins["engine"],
                                "ins": [],
                                "outs": [],
                                "name": f"waitfix-{ctr}",
                                "opcode": "NoOp",
                                "sync_info": {"on_wait": [w], "on_update": []},
                            }
                        )
                    si["on_wait"] = [waits[-1]]
                out.append(ins)
            blk["instructions"] = out
    return json.dumps(j).encode()


def _build():
    import concourse.bass as bass
    import concourse.mybir as mybir
    import concourse.tile as tile

    f32 = mybir.dt.float32
    bf16 = mybir.dt.bfloat16
    AF = mybir.ActivationFunctionType
    ALU = mybir.AluOpType

    nc = bass.Bass(
        "TRN2", target_bir_lowering=False, debug=False, num_devices=NCORES
    )

    # activations/weights arrive pre-arranged on host into the exact SBUF
    # layout ([partition, ...] contiguous) so every load is a 1:1 DMA with
    # 2KB+ lines and 128 descriptors.
    qT = [nc.dram_tensor(f"qT{b}", [128, NTQB * NDC * XB], bf16, kind="ExternalInput") for b in range(B)]
    kT = [nc.dram_tensor(f"kT{b}", [128, NTQB * NDC * XB], bf16, kind="ExternalInput") for b in range(B)]
    vT = [nc.dram_tensor(f"vT{b}", [128, NTQB * NDC * XB], bf16, kind="ExternalInput") for b in range(B)]
    wq = nc.dram_tensor("wq", [128, NDC * NW], bf16, kind="ExternalInput")
    wk = nc.dram_tensor("wk", [128, NDC * NW], bf16, kind="ExternalInput")
    wv = nc.dram_tensor("wv", [128, NDC * NW], bf16, kind="ExternalInput")
    wp = nc.dram_tensor("wp", [128, (N_HEADS * H // 128) * D], bf16, kind="ExternalInput")
    bq = nc.dram_tensor("bq", [128, 1], f32, kind="ExternalInput")
    bk = nc.dram_tensor("bk", [128, 1], f32, kind="ExternalInput")
    bv = nc.dram_tensor("bv", [128, 1], f32, kind="ExternalInput")
    bp = nc.dram_tensor("bp", [128, D], f32, kind="ExternalInput")
    ident = nc.dram_tensor("ident", [128, 128], bf16, kind="ExternalInput")
    identf = nc.dram_tensor("identf", [128, 128], f32, kind="ExternalInput")
    out = nc.dram_tensor("out", [B * SLICE, D], f32, kind="ExternalOutput")

    with tile.TileContext(nc) as tc, ExitStack() as ctx:
        p_const = ctx.enter_context(tc.tile_pool(name="const", bufs=1))
        p_x = ctx.enter_context(tc.tile_pool(name="x", bufs=1))
        p_qk = ctx.enter_context(tc.tile_pool(name="qk", bufs=2))
        p_va = ctx.enter_context(tc.tile_pool(name="va", bufs=2))
        p_pt = ctx.enter_context(tc.tile_pool(name="pt", bufs=2))
        p_a = ctx.enter_context(tc.tile_pool(name="a", bufs=2))
        p_o = ctx.enter_context(tc.tile_pool(name="o", bufs=2))
        p_dram = ctx.enter_context(tc.tile_pool(name="dram", bufs=1, space="DRAM"))

        ps_ss = ctx.enter_context(tc.tile_pool(name="ps_ss", bufs=1, space="PSUM"))
        ps_pv = ctx.enter_context(tc.tile_pool(name="ps_pv", bufs=1, space="PSUM"))
        ps_mm = ctx.enter_context(tc.tile_pool(name="ps_mm", bufs=2, space="PSUM"))

        # ---- constant loads -------------------------------------------------
        wq_sb = p_const.tile([128, NDC * NW], bf16)
        wk_sb = p_const.tile([128, NDC * NW], bf16)
        wv_sb = p_const.tile([128, NDC * NW], bf16)
        wp_sb = p_const.tile([128, (N_HEADS * H // 128) * D], bf16)
        bq_sb = p_const.tile([128, 1], f32)
        bk_sb = p_const.tile([128, 1], f32)
        bv_sb = p_const.tile([128, 1], f32)
        bp_sb = p_const.tile([128, D], f32)
        id_sb = p_const.tile([128, 128], bf16)
        idf_sb = p_const.tile([128, 128], f32)
        nc.sync.dma_start(id_sb[:], ident[:])
        nc.sync.dma_start(idf_sb[:], identf[:])
        for sb_t, ext in (
            (wq_sb, wq),
            (wk_sb, wk),
            (wv_sb, wv),
            (wp_sb, wp),
            (bq_sb, bq),
            (bk_sb, bk),
            (bv_sb, bv),
            (bp_sb, bp),
        ):
            nc.sync.dma_start(sb_t[:], ext[:])

        # warm the ACT exp table while everything else is still loading
        warm = p_const.tile([1, 8], bf16)
        nc.vector.memset(warm[:], 0.0)
        nc.scalar.activation(warm[:], warm[:], AF.Exp)

        # f32 ones, used as K=1 transpose "identity" and as the ones column
        # of the reciprocal-row broadcast outer product
        onesf = p_const.tile([128, H], f32)
        nc.vector.memset(onesf[:], 1.0)

        a2a_in = [p_dram.tile([NCORES * NW, SLICE], bf16, name=f"a2ai{b}") for b in range(B)]
        a2a_out = [p_dram.tile([NCORES * NW, SLICE], bf16, name=f"a2ao{b}") for b in range(B)]

        # ---- x loads: per (tensor, tb) contiguous 8KB-line chunks -----------
        def load_x(b):
            xs = {}
            CW = NDC * XB  # 4096 cols per tb chunk
            for tname, ext in (("v", vT[b]), ("k", kT[b]), ("q", qT[b])):
                for tb in range(NTQB):
                    t_ = p_x.tile(
                        [128, CW], bf16,
                        name=f"x{tname}{b}{tb}", tag=f"x{tname}{tb}",
                    )
                    nc.sync.dma_start(t_[:], ext[:, tb * CW : (tb + 1) * CW])
                    xs[(tname, tb)] = t_
            return xs

        # ---- projection pieces (emitted inline or as PE fillers) ------------
        def proj_block(w_sb, b_sb, xt, dst, dcol, b, tag):
            ps = ps_mm.tile([128, XB], f32, name=f"mm{tag}{b}{dcol}", tag="mm")
            for dc in range(NDC):
                nc.tensor.matmul(
                    ps[:],
                    lhsT=w_sb[:, dc * NW : (dc + 1) * NW],
                    rhs=xt[:, dc * XB : (dc + 1) * XB],
                    start=(dc == 0),
                    stop=(dc == NDC - 1),
                )
            nc.vector.tensor_scalar(
                dst[:, dcol * XB : (dcol + 1) * XB], ps[:], b_sb[:, 0:1], None,
                ALU.add,
            )

        def va_piece(b, vt, va, i0, n):
            """Transpose tk-chunks [i0, i0+n) of vt into va."""
            for i in range(i0, i0 + n):
                pst = ps_mm.tile([128, 128], bf16, name=f"pst{b}{i}", tag="mm")
                nc.tensor.transpose(pst[:], vt[:, i * TKC : (i + 1) * TKC], id_sb[:])
                dst = va[:, i * NLOC * VA : (i + 1) * NLOC * VA].rearrange(
                    "p (h x) -> p h x", x=VA
                )[:, :, 0:H]
                nc.vector.tensor_copy(dst, pst[:].rearrange("p (h x) -> p h x", x=H))

        def make_proj(b, xs):
            """Returns (qt, kt, va, fillers): fillers produce vt/va/kt and
            qt blocks 1..3; caller must run the first pieces inline for b=0."""
            vt = p_va.tile([128, T], bf16, name=f"vt{b}", tag="vt")
            va = p_va.tile([128, NTKC * NLOC * VA], bf16, name=f"va{b}", tag="va")
            qt = p_qk.tile([128, T], bf16, name=f"qt{b}", tag="qt")
            kt = p_qk.tile([128, T], bf16, name=f"kt{b}", tag="kt")
            nc.vector.memset(
                va[:].rearrange("p (i h x) -> p i h x", h=NLOC, x=VA)[
                    :, :, :, H : H + 1
                ],
                1.0,
            )
            pieces = []
            for tb in range(NTQB):
                pieces.append(
                    lambda tb=tb: proj_block(wv_sb, bv_sb, xs[("v", tb)], vt, tb, b, "v")
                )
            for i0 in range(0, NTKC, 4):
                pieces.append(lambda i0=i0: va_piece(b, vt, va, i0, 4))
            for tb in range(NTQB):
                pieces.append(
                    lambda tb=tb: proj_block(wk_sb, bk_sb, xs[("k", tb)], kt, tb, b, "k")
                )
            for tb in range(NTQB):
                pieces.append(
                    lambda tb=tb: proj_block(wq_sb, bq_sb, xs[("q", tb)], qt, tb, b, "q")
                )
            return qt, kt, va, pieces

        def outproj_pieces(b):
            """Output projection for this core's 256-row slice of batch b."""
            ats = p_a.tile([128, NCORES * SLICE], bf16, name=f"ats{b}", tag="ats")

            def load():
                nc.sync.dma_start(
                    ats[:].rearrange("p (c t) -> p c t", t=SLICE),
                    a2a_out[b][:].rearrange("(c p) t -> p c t", p=128),
                )

            pieces = [load]
            for tqc in range(SLICE // 128):
                for dh in range(2):
                    def op(tqc=tqc, dh=dh):
                        ps = ps_mm.tile(
                            [128, 512], f32, name=f"po{b}{tqc}{dh}", tag="mm"
                        )
                        for nhc in range(NCORES):
                            nc.tensor.matmul(
                                ps[:],
                                lhsT=ats[:, nhc * SLICE + tqc * 128 : nhc * SLICE + (tqc + 1) * 128],
                                rhs=wp_sb[:, nhc * D + dh * 512 : nhc * D + (dh + 1) * 512],
                                start=(nhc == 0),
                                stop=(nhc == NCORES - 1),
                            )
                        o_sb = p_o.tile(
                            [128, 512], f32, name=f"o{b}{tqc}{dh}", tag="o"
                        )
                        nc.vector.tensor_tensor(
                            o_sb[:], ps[:], bp_sb[:, dh * 512 : (dh + 1) * 512],
                            ALU.add,
                        )
                        nc.sync.dma_start(
                            out[
                                b * SLICE + tqc * 128 : b * SLICE + (tqc + 1) * 128,
                                dh * 512 : (dh + 1) * 512,
                            ],
                            o_sb[:],
                        )
                    pieces.append(op)
            return pieces

        fillers = []

        def run_filler(n=1):
            for _ in range(n):
                if fillers:
                    fillers.pop(0)()

        def make_norm(b, j, pv):
            """Normalize + stage block (b, j). Emitted one group into the
            NEXT block so the PE pipeline never drains at block boundaries.
            The softmax denominator row [1, 512] is PE-transposed to [128, 4]
            so the DVE reciprocal is fast (free dim 4, not 512), transposed
            back to a row, and outer-product-broadcast to 64 partitions."""
            NTR = TQB // 128  # 4

            def norm():
                ans = []
                for hd in range(NLOC):
                    a_sb = p_a.tile(
                        [H + 1, TQB], f32, name=f"as{b}{j}{hd}", tag=f"as{hd}"
                    )
                    nc.vector.tensor_copy(a_sb[:], pv[hd][0 : H + 1, :])
                    trp = ps_mm.tile([128, NTR], f32, name=f"trp{b}{j}{hd}", tag="mm")
                    for i in range(NTR):
                        nc.tensor.transpose(
                            trp[:, i : i + 1],
                            a_sb[H : H + 1, i * 128 : (i + 1) * 128],
                            onesf[H : H + 1, 0:1],
                        )
                    rc = p_a.tile([128, NTR], f32, name=f"rc{b}{j}{hd}", tag="rc")
                    nc.vector.reciprocal(rc[:], trp[:])
                    rowt = ps_mm.tile([1, TQB], f32, name=f"rw{b}{j}{hd}", tag="mm")
                    for i in range(NTR):
                        nc.tensor.transpose(
                            rowt[:, i * 128 : (i + 1) * 128],
                            rc[:, i : i + 1],
                            idf_sb[:, 0:128],
                        )
                    rr = p_a.tile([1, TQB], f32, name=f"rr{b}{j}{hd}", tag="rr")
                    nc.vector.tensor_copy(rr[:], rowt[:])
                    rep_ps = ps_mm.tile([H, TQB], f32, name=f"rp{b}{j}{hd}", tag="mm")
                    nc.tensor.matmul(
                        rep_ps[:],
                        lhsT=onesf[0:1, 0:H],
                        rhs=rr[:],
                        start=True,
                        stop=True,
                    )
                    an = p_a.tile([H, TQB], bf16, name=f"an{b}{j}{hd}", tag=f"an{hd}")
                    nc.vector.tensor_tensor(
                        an[:], a_sb[0:H, :], rep_ps[:], ALU.mult
                    )
                    ans.append(an)
                for half in range(2):
                    s = 2 * j + half
                    for hd in range(NLOC):
                        nc.sync.dma_start(
                            a2a_in[b][s * NW + hd * H : s * NW + (hd + 1) * H, :],
                            ans[hd][:, half * SLICE : (half + 1) * SLICE],
                        )

            return norm

        def attn_block(b, j, qt, kt, va, pending, fill_groups=None):
            """One (batch, tq-block) attention block, software-pipelined:
            scores(g+1) are emitted before PV(g) so the PE keeps busy while
            ACT exps group g; filler pieces run after selected groups.
            Returns this block's normalize closure; the previous block's
            (`pending`) is emitted after the first score/exp group."""
            pv = [
                ps_pv.tile([VA, TQB], f32, name=f"pv{b}{j}{hd}", tag=f"pv{hd}")
                for hd in range(NLOC)
            ]
            pss = {}
            pt = {}

            def scores(g):
                for hd in range(NLOC):
                    pss[(g % 2, hd)] = ps_ss.tile(
                        [128, RG * TQB], f32,
                        name=f"pss{b}{j}{g}{hd}", tag=f"ss{hd}",
                    )
                for r in range(g * RG, (g + 1) * RG):
                    for hd in range(NLOC):
                        nc.tensor.matmul(
                            pss[(g % 2, hd)][:, (r % RG) * TQB : (r % RG + 1) * TQB],
                            lhsT=kt[hd * H : (hd + 1) * H, r * TKC : (r + 1) * TKC],
                            rhs=qt[hd * H : (hd + 1) * H, j * TQB : (j + 1) * TQB],
                            start=True,
                            stop=True,
                        )

            def exps(g):
                for hd in range(NLOC):
                    pt[(g % 2, hd)] = p_pt.tile(
                        [128, RG * TQB], bf16,
                        name=f"pt{b}{j}{g}{hd}", tag=f"pt{hd}",
                    )
                    nc.scalar.activation(
                        pt[(g % 2, hd)][:], pss[(g % 2, hd)][:], AF.Exp
                    )

            def pvs(g):
                for hd in range(NLOC):
                    for r in range(g * RG, (g + 1) * RG):
                        col0 = (r * NLOC + hd) * VA
                        nc.tensor.matmul(
                            pv[hd][:],
                            lhsT=va[:, col0 : col0 + VA],
                            rhs=pt[(g % 2, hd)][:, (r % RG) * TQB : (r % RG + 1) * TQB],
                            start=(r == g * RG and g == 0),
                            stop=(r == (g + 1) * RG - 1 and g == NG - 1),
                        )

            scores(0)
            exps(0)
            for g in range(NG):
                if g + 1 < NG:
                    scores(g + 1)
                    exps(g + 1)
                if g == 0 and pending is not None:
                    pending()
                pvs(g)
                if fill_groups is None or g in fill_groups:
                    run_filler()
            return make_norm(b, j, pv)

        # ===== main schedule =================================================
        xs0 = load_x(0)
        qt0, kt0, va0, pieces0 = make_proj(0, xs0)
        # inline: V, va, K, Q(tb=0); leave Q(tb 1..3) as fillers
        for p in pieces0[:13]:
            p()
        fillers.extend(pieces0[13:])

        xs1 = load_x(1)
        qt1, kt1, va1, pieces1 = make_proj(1, xs1)
        fillers.extend(pieces1)

        pending = None
        for j in range(NTQB):
            pending = attn_block(0, j, qt0, kt0, va0, pending)
        pending()
        pending = None
        nc.gpsimd.collective_compute(
            "AllToAll",
            ALU.bypass,
            replica_groups=GROUPS,
            ins=[a2a_in[0].opt()],
            outs=[a2a_out[0].opt()],
        )
        fillers.extend(outproj_pieces(0))

        # pop fillers only on groups 3/7 during b=1: the b=0 output
        # projection pieces then land ~35us+ after the b=0 AllToAll was
        # issued, so the PE does not stall waiting for it.
        for j in range(NTQB):
            pending = attn_block(1, j, qt1, kt1, va1, pending, fill_groups=(3, 7))
        pending()
        nc.gpsimd.collective_compute(
            "AllToAll",
            ALU.bypass,
            replica_groups=GROUPS,
            ins=[a2a_in[1].opt()],
            outs=[a2a_out[1].opt()],
        )
        run_filler(len(fillers))
        for p in outproj_pieces(1):
            p()

    orig_to_json = nc.to_json_bytes
    nc.to_json_bytes = lambda: _legalize_waits(orig_to_json())
    return nc


def _get_nc():
    if "nc" not in _CACHE:
        _CACHE["nc"] = _build()
    return _CACHE["nc"]


def _make_in_maps(inputs):
    q = np.asarray(inputs["q"], dtype=np.float32)
    v = np.asarray(inputs["v"], dtype=np.float32)
    k = np.asarray(inputs["k"], dtype=np.float32)
    w_query = np.asarray(inputs["w_query"], dtype=np.float32)
    b_query = np.asarray(inputs["b_query"], dtype=np.float32)
    w_value = np.asarray(inputs["w_value"], dtype=np.float32)
    b_value = np.asarray(inputs["b_value"], dtype=np.float32)
    w_key = np.asarray(inputs["w_key"], dtype=np.float32)
    b_key = np.asarray(inputs["b_key"], dtype=np.float32)
    w_projection = np.asarray(inputs["w_projection"], dtype=np.float32)
    b_projection = np.asarray(inputs["b_projection"], dtype=np.float32)

    scale = np.float32(1.0 / np.sqrt(H))

    def arrange_w(w):
        # [D or N*H, m] -> SBUF layout [128, (chunk, m)], contiguous rows
        m = w.shape[1]
        return np.ascontiguousarray(
            w.reshape(-1, 128, m).transpose(1, 0, 2).reshape(128, -1)
        ).astype(BF16)

    def arrange_x(xb):
        # [T, D] -> [D, T] -> SBUF layout [128, (tb, dc, t)], contiguous rows
        return np.ascontiguousarray(
            xb.T.reshape(NDC, 128, NTQB, XB)
            .transpose(1, 2, 0, 3)
            .reshape(128, NTQB * NDC * XB)
        ).astype(BF16)

    wp_s = arrange_w(
        np.ascontiguousarray(
            w_projection.transpose(0, 2, 1).reshape(N_HEADS * H, D)
        )
    )
    bp_s = np.ascontiguousarray(
        np.tile(b_projection.reshape(1, D), (128, 1))
    ).astype(np.float32)

    xT = {}
    for b in range(B):
        xT[b] = tuple(arrange_x(x[b]) for x in (q, k, v))

    in_maps = []
    for c in range(NCORES):
        hs = c * NLOC
        wq_s = arrange_w(w_query[:, hs : hs + NLOC, :].reshape(D, NW) * scale)
        wk_s = arrange_w(w_key[:, hs : hs + NLOC, :].reshape(D, NW))
        wv_s = arrange_w(w_value[:, hs : hs + NLOC, :].reshape(D, NW))
        bq_s = np.ascontiguousarray(
            (b_query[hs : hs + NLOC].reshape(NW) * scale).reshape(NW, 1)
        ).astype(np.float32)
        bk_s = np.ascontiguousarray(
            b_key[hs : hs + NLOC].reshape(NW, 1)
        ).astype(np.float32)
        bv_s = np.ascontiguousarray(
            b_value[hs : hs + NLOC].reshape(NW, 1)
        ).astype(np.float32)
        m = {
            "ident": np.eye(128, dtype=np.float32).astype(BF16),
            "identf": np.eye(128, dtype=np.float32),
            "wq": np.ascontiguousarray(wq_s),
            "wk": np.ascontiguousarray(wk_s),
            "wv": np.ascontiguousarray(wv_s),
            "wp": wp_s,
            "bq": bq_s,
            "bk": bk_s,
            "bv": bv_s,
            "bp": bp_s,
        }
        for b in range(B):
            m[f"qT{b}"], m[f"kT{b}"], m[f"vT{b}"] = xT[b]
        in_maps.append(m)
    return in_maps


def _assemble(results):
    out = np.empty((B, T, D), np.float32)
    for c in range(NCORES):
        res = results[c]["out"]
        for b in range(B):
            out[b, c * SLICE : (c + 1) * SLICE, :] = res[
                b * SLICE : (b + 1) * SLICE
            ]
    return out


def run(inputs, trace=False, **kwargs):
    from concourse.bass_utils import run_bass_kernel_spmd

    nc = _get_nc()
    in_maps = _make_in_maps(inputs)
    res = run_bass_kernel_spmd(
        nc, in_maps, list(range(NCORES)), trace=trace, **kwargs
    )
    return _assemble(res.results), res


def kernel(**inputs) -> np.ndarray:
    out, _ = run(inputs, trace=False)
    return out


# revision 28
# speedup vs baseline: 1.2163x; 1.2087x over previous
"""Bass/Tile TRN2 kernel: 16-head MHA (B=2, T=2048, D=1024, H=64) on 8 NeuronCores.

Sharding: 8-way tensor parallel over heads — core c computes heads {2c, 2c+1}
for BOTH batches. Output sharding is (batch, 256-row tq slice): after each
batch's attention, one AllToAll swaps head-shards for tq-slice shards and the
core runs the output projection for its 256-row slice of that batch. The b=0
AllToAll + output projection overlap b=1's attention; only b=1's 512KB
collective is exposed at the tail.

Per-core pipeline (everything bf16 into the PE, fp32 PSUM accumulation):
  - Inputs arrive pre-transposed ([D, T]) and are DMA'd in 512-column chunks
    so the first V-projection matmul issues ~4us in.
  - QKV projections: 8x [128,128]x[128,512] accumulating matmuls per block.
  - Scores S^T[tk, tq] = K^T.T @ Q^T per head, written to PSUM as bf16 so a
    [128, 2048] score tile (4 tk-chunks of one head) is 2 banks; 1/sqrt(H) is
    folded into Wq/bq on host.
  - exp on ScalarE straight out of PSUM in 2048-wide ACTIVATEs (ACT is the
    critical engine: ~(N+352)/1.2ns, dtype-independent). Per-head score tiles
    ping-pong so ACT never waits on score matmuls.
  - PV matmul with a ones-augmented V (stationary col 64 = ones) so row 64 of
    the PV accumulator is the softmax denominator for free.
  - Normalize: DVE reciprocal of the denominator row, GPSIMD
    partition-broadcast to 64 rows, DVE multiply (psum read) -> staged shard.
  - PE idle slots inside the ACT-bound attention phase are filled with the
    next batch's projections and the previous batch's output projection
    (software pipelining keeps the PE HAM-warm at 2.4 GHz).
Host does layout-only prep (transpose, bf16 cast, weight slicing) and
concatenates the 8 cores' 2x256-row output slices.
"""

import sys
from contextlib import ExitStack

import numpy as np

sys.path.insert(0, "/opt/trn_rl_repo")

import ml_dtypes  # noqa: E402

BF16 = ml_dtypes.bfloat16

B, T, D = 2, 2048, 1024
N_HEADS, H = 16, 64
NCORES = 8
GROUPS = [[0, 1, 2, 3, 4, 5, 6, 7]]
NLOC = 2            # heads per core
TQB = 512           # attention tq block
NTQB = T // TQB     # 4
TKC = 128           # tk chunk
NTKC = T // TKC     # 16
RG = 2              # tk chunks per exp group
NG = NTKC // RG     # 8 groups per block
DC = 128            # d chunk
NDC = D // DC       # 8
XB = 512            # x-load column block
SLICE = 256         # output tq rows per core per batch
VA = 128            # V_aug stationary width: [V(64) | ones(1) | junk(63)]
NW = NLOC * H       # 128 projection width per core

_CACHE = {}


def _legalize_waits(bir_bytes):
    """This toolchain's walrus accepts at most ONE semaphore wait per
    instruction ("Too many sync wait commands"). Tile's sem assignment emits
    several. Hoist all but one wait of each instruction onto same-engine NoOps
    inserted immediately before it (engines execute their stream in order, so
    waiting earlier on the same engine is equivalent)."""
    import json

    j = json.loads(bir_bytes)
    ctr = 0
    for fn in j["functions"]:
        for blk in fn["blocks"]:
            out = []
            for ins in blk["instructions"]:
                si = ins.get("sync_info")
                waits = (si or {}).get("on_wait") or []
                if len(waits) > 1:
                    for w in waits[:-1]:
                        ctr += 1
                        out.append(
                            {
                                "engine": ins["engine"],
                                "ins": [],
                                "outs": [],
                                "name": f"waitfix-{ctr}",
                                "opcode": "NoOp",
                                "sync_info": {"on_wait": [w], "on_update": []},
                            }
                        )
                    si["on_wait"] = [waits[-1]]
                out.append(ins)
            blk["instructions"] = out
    return json.dumps(j).encode()


def _build():
    import concourse.bass as bass
    import concourse.mybir as mybir
    import concourse.tile as tile

    f32 = mybir.dt.float32
    bf16 = mybir.dt.bfloat16
    AF = mybir.ActivationFunctionType
    ALU = mybir.AluOpType

    nc = bass.Bass(
        "TRN2", target_bir_lowering=False, debug=False, num_devices=NCORES
    )

    # activations/weights arrive pre-arranged on host into the exact SBUF
    # layout ([partition, ...] contiguous) so every load is a 1:1 DMA with
    # 2KB+ lines and 128 descriptors.
    qT = [nc.dram_tensor(f"qT{b}", [128, NTQB * NDC * XB], bf16, kind="ExternalInput") for b in range(B)]
    kT = [nc.dram_tensor(f"kT{b}", [128, NTQB * NDC * XB], bf16, kind="ExternalInput") for b in range(B)]
    vT = [nc.dram_tensor(f"vT{b}", [128, NTQB * NDC * XB], bf16, kind="ExternalInput") for b in range(B)]
    wq = nc.dram_tensor("wq", [128, NDC * NW], bf16, kind="ExternalInput")
    wk = nc.dram_tensor("wk", [128, NDC * NW], bf16, kind="ExternalInput")
    wv = nc.dram_tensor("wv", [128, NDC * NW], bf16, kind="ExternalInput")
    wp = nc.dram_tensor("wp", [128, (N_HEADS * H // 128) * D], bf16, kind="ExternalInput")
    bq = nc.dram_tensor("bq", [128, 1], f32, kind="ExternalInput")
    bk = nc.dram_tensor("bk", [128, 1], f32, kind="ExternalInput")
    bv = nc.dram_tensor("bv", [128, 1], f32, kind="ExternalInput")
    bp = nc.dram_tensor("bp", [128, D], f32, kind="ExternalInput")
    ident = nc.dram_tensor("ident", [128, 128], bf16, kind="ExternalInput")
    identf = nc.dram_tensor("identf", [128, 128], f32, kind="ExternalInput")
    out = nc.dram_tensor("out", [B * SLICE, D], f32, kind="ExternalOutput")

    with tile.TileContext(nc) as tc, ExitStack() as ctx:
        p_const = ctx.enter_context(tc.tile_pool(name="const", bufs=1))
        p_x = ctx.enter_context(tc.tile_pool(name="x", bufs=1))
        p_qk = ctx.enter_context(tc.tile_pool(name="qk", bufs=2))
        p_va = ctx.enter_context(tc.tile_pool(name="va", bufs=2))
        p_pt = ctx.enter_context(tc.tile_pool(name="pt", bufs=2))
        p_a = ctx.enter_context(tc.tile_pool(name="a", bufs=2))
        p_o = ctx.enter_context(tc.tile_pool(name="o", bufs=2))
        p_dram = ctx.enter_context(tc.tile_pool(name="dram", bufs=1, space="DRAM"))

        ps_ss = ctx.enter_context(tc.tile_pool(name="ps_ss", bufs=1, space="PSUM"))
        ps_pv = ctx.enter_context(tc.tile_pool(name="ps_pv", bufs=1, space="PSUM"))
        ps_mm = ctx.enter_context(tc.tile_pool(name="ps_mm", bufs=2, space="PSUM"))

        # ---- constant loads -------------------------------------------------
        wq_sb = p_const.tile([128, NDC * NW], bf16)
        wk_sb = p_const.tile([128, NDC * NW], bf16)
        wv_sb = p_const.tile([128, NDC * NW], bf16)
        wp_sb = p_const.tile([128, (N_HEADS * H // 128) * D], bf16)
        bq_sb = p_const.tile([128, 1], f32)
        bk_sb = p_const.tile([128, 1], f32)
        bv_sb = p_const.tile([128, 1], f32)
        bp_sb = p_const.tile([128, D], f32)
        id_sb = p_const.tile([128, 128], bf16)
        idf_sb = p_const.tile([128, 128], f32)

        # warm the ACT exp table while everything else is still loading
        warm = p_const.tile([1, 8], bf16)
        nc.vector.memset(warm[:], 0.0)
        nc.scalar.activation(warm[:], warm[:], AF.Exp)

        # f32 ones, used as K=1 transpose "identity" and as the ones column
        # of the reciprocal-row broadcast outer product
        onesf = p_const.tile([128, H], f32)
        nc.vector.memset(onesf[:], 1.0)

        a2a_in = [p_dram.tile([NCORES * NW, SLICE], bf16, name=f"a2ai{b}") for b in range(B)]
        a2a_out = [p_dram.tile([NCORES * NW, SLICE], bf16, name=f"a2ao{b}") for b in range(B)]

        # ---- x loads: per (tensor, tb) contiguous 8KB-line chunks -----------
        CW = NDC * XB  # 4096 cols per tb chunk
        exts = {"v": vT, "k": kT, "q": qT}

        def load_x_one(b, tname, tb, xs):
            t_ = p_x.tile(
                [128, CW], bf16, name=f"x{tname}{b}{tb}", tag=f"x{tname}{tb}"
            )
            nc.sync.dma_start(t_[:], exts[tname][b][:, tb * CW : (tb + 1) * CW])
            xs[(tname, tb)] = t_

        def load_x(b):
            xs = {}
            for tname in ("v", "k", "q"):
                for tb in range(NTQB):
                    load_x_one(b, tname, tb, xs)
            return xs

        def load_startup():
            """b=0 loads interleaved with weights in consumption order, so
            the V projection starts ~5us in; wp (2MB, needed ~150us in)
            loads last."""
            xs = {}
            nc.sync.dma_start(wv_sb[:], wv[:])
            nc.sync.dma_start(bv_sb[:], bv[:])
            nc.sync.dma_start(id_sb[:], ident[:])
            for tb in range(NTQB):
                load_x_one(0, "v", tb, xs)
            nc.sync.dma_start(wk_sb[:], wk[:])
            nc.sync.dma_start(bk_sb[:], bk[:])
            for tb in range(NTQB):
                load_x_one(0, "k", tb, xs)
            nc.sync.dma_start(wq_sb[:], wq[:])
            nc.sync.dma_start(bq_sb[:], bq[:])
            for tb in range(NTQB):
                load_x_one(0, "q", tb, xs)
            nc.sync.dma_start(idf_sb[:], identf[:])
            nc.sync.dma_start(bp_sb[:], bp[:])
            nc.sync.dma_start(wp_sb[:], wp[:])
            return xs

        # ---- projection pieces (emitted inline or as PE fillers) ------------
        def proj_block(w_sb, b_sb, xt, dst, dcol, b, tag):
            ps = ps_mm.tile([128, XB], f32, name=f"mm{tag}{b}{dcol}", tag="mm")
            for dc in range(NDC):
                nc.tensor.matmul(
                    ps[:],
                    lhsT=w_sb[:, dc * NW : (dc + 1) * NW],
                    rhs=xt[:, dc * XB : (dc + 1) * XB],
                    start=(dc == 0),
                    stop=(dc == NDC - 1),
                )
            nc.vector.tensor_scalar(
                dst[:, dcol * XB : (dcol + 1) * XB], ps[:], b_sb[:, 0:1], None,
                ALU.add,
            )

        def va_piece(b, vt, va, i0, n):
            """Transpose tk-chunks [i0, i0+n) of vt into va."""
            for i in range(i0, i0 + n):
                pst = ps_mm.tile([128, 128], bf16, name=f"pst{b}{i}", tag="mm")
                nc.tensor.transpose(pst[:], vt[:, i * TKC : (i + 1) * TKC], id_sb[:])
                dst = va[:, i * NLOC * VA : (i + 1) * NLOC * VA].rearrange(
                    "p (h x) -> p h x", x=VA
                )[:, :, 0:H]
                nc.vector.tensor_copy(dst, pst[:].rearrange("p (h x) -> p h x", x=H))

        def make_proj(b, xs):
            """Returns (qt, kt, va, fillers): fillers produce vt/va/kt and
            qt blocks 1..3; caller must run the first pieces inline for b=0."""
            vt = p_va.tile([128, T], bf16, name=f"vt{b}", tag="vt")
            va = p_va.tile([128, NTKC * NLOC * VA], bf16, name=f"va{b}", tag="va")
            qt = p_qk.tile([128, T], bf16, name=f"qt{b}", tag="qt")
            kt = p_qk.tile([128, T], bf16, name=f"kt{b}", tag="kt")
            nc.vector.memset(
                va[:].rearrange("p (i h x) -> p i h x", h=NLOC, x=VA)[
                    :, :, :, H : H + 1
                ],
                1.0,
            )
            pieces = []
            for tb in range(NTQB):
                pieces.append(
                    lambda tb=tb: proj_block(wv_sb, bv_sb, xs[("v", tb)], vt, tb, b, "v")
                )
            for i0 in range(0, NTKC, 4):
                pieces.append(lambda i0=i0: va_piece(b, vt, va, i0, 4))
            for tb in range(NTQB):
                pieces.append(
                    lambda tb=tb: proj_block(wk_sb, bk_sb, xs[("k", tb)], kt, tb, b, "k")
                )
            for tb in range(NTQB):
                pieces.append(
                    lambda tb=tb: proj_block(wq_sb, bq_sb, xs[("q", tb)], qt, tb, b, "q")
                )
            return qt, kt, va, pieces

        def outproj_pieces(b):
            """Output projection for this core's 256-row slice of batch b."""
            ats = p_a.tile([128, NCORES * SLICE], bf16, name=f"ats{b}", tag="ats")

            def load():
                nc.sync.dma_start(
                    ats[:].rearrange("p (c t) -> p c t", t=SLICE),
                    a2a_out[b][:].rearrange("(c p) t -> p c t", p=128),
                )

            pieces = [load]
            for tqc in range(SLICE // 128):
                for dh in range(2):
                    def op(tqc=tqc, dh=dh):
                        ps = ps_mm.tile(
                            [128, 512], f32, name=f"po{b}{tqc}{dh}", tag="mm"
                        )
                        for nhc in range(NCORES):
                            nc.tensor.matmul(
                                ps[:],
                                lhsT=ats[:, nhc * SLICE + tqc * 128 : nhc * SLICE + (tqc + 1) * 128],
                                rhs=wp_sb[:, nhc * D + dh * 512 : nhc * D + (dh + 1) * 512],
                                start=(nhc == 0),
                                stop=(nhc == NCORES - 1),
                            )
                        o_sb = p_o.tile(
                            [128, 512], f32, name=f"o{b}{tqc}{dh}", tag="o"
                        )
                        nc.vector.tensor_tensor(
                            o_sb[:], ps[:], bp_sb[:, dh * 512 : (dh + 1) * 512],
                            ALU.add,
                        )
                        nc.sync.dma_start(
                            out[
                                b * SLICE + tqc * 128 : b * SLICE + (tqc + 1) * 128,
                                dh * 512 : (dh + 1) * 512,
                            ],
                            o_sb[:],
                        )
                    pieces.append(op)
            return pieces

        fillers = []

        def run_filler(n=1):
            for _ in range(n):
                if fillers:
                    fillers.pop(0)()

        def make_norm(b, j, pv):
            """Normalize + stage block (b, j). Emitted one group into the
            NEXT block so the PE pipeline never drains at block boundaries.
            The softmax denominator row [1, 512] is PE-transposed to [128, 4]
            so the DVE reciprocal is fast (free dim 4, not 512), transposed
            back to a row, and outer-product-broadcast to 64 partitions."""
            NTR = TQB // 128  # 4

            def norm():
                ans = []
                for hd in range(NLOC):
                    a_sb = p_a.tile(
                        [H + 1, TQB], f32, name=f"as{b}{j}{hd}", tag=f"as{hd}"
                    )
                    nc.vector.tensor_copy(a_sb[:], pv[hd][0 : H + 1, :])
                    trp = ps_mm.tile([128, NTR], f32, name=f"trp{b}{j}{hd}", tag="mm")
                    for i in range(NTR):
                        nc.tensor.transpose(
                            trp[:, i : i + 1],
                            a_sb[H : H + 1, i * 128 : (i + 1) * 128],
                            onesf[H : H + 1, 0:1],
                        )
                    rc = p_a.tile([128, NTR], f32, name=f"rc{b}{j}{hd}", tag="rc")
                    nc.vector.reciprocal(rc[:], trp[:])
                    rowt = ps_mm.tile([1, TQB], f32, name=f"rw{b}{j}{hd}", tag="mm")
                    for i in range(NTR):
                        nc.tensor.transpose(
                            rowt[:, i * 128 : (i + 1) * 128],
                            rc[:, i : i + 1],
                            idf_sb[:, 0:128],
                        )
                    rr = p_a.tile([1, TQB], f32, name=f"rr{b}{j}{hd}", tag="rr")
                    nc.vector.tensor_copy(rr[:], rowt[:])
                    rep_ps = ps_mm.tile([H, TQB], f32, name=f"rp{b}{j}{hd}", tag="mm")
                    nc.tensor.matmul(
                        rep_ps[:],
                        lhsT=onesf[0:1, 0:H],
                        rhs=rr[:],
                        start=True,
                        stop=True,
                    )
                    an = p_a.tile([H, TQB], bf16, name=f"an{b}{j}{hd}", tag=f"an{hd}")
                    nc.vector.tensor_tensor(
                        an[:], a_sb[0:H, :], rep_ps[:], ALU.mult
                    )
                    ans.append(an)
                for half in range(2):
                    s = 2 * j + half
                    for hd in range(NLOC):
                        nc.sync.dma_start(
                            a2a_in[b][s * NW + hd * H : s * NW + (hd + 1) * H, :],
                            ans[hd][:, half * SLICE : (half + 1) * SLICE],
                        )

            return norm

        def attn_block(b, j, qt, kt, va, fill_groups=None):
            """One (batch, tq-block) attention block, software-pipelined:
            scores(g+1) are emitted before PV(g) so the PE keeps busy while
            ACT exps group g; filler pieces run after selected groups."""
            pv = [
                ps_pv.tile([VA, TQB], f32, name=f"pv{b}{j}{hd}", tag=f"pv{hd}")
                for hd in range(NLOC)
            ]
            pss = {}
            pt = {}

            def scores(g):
                for hd in range(NLOC):
                    pss[(g % 2, hd)] = ps_ss.tile(
                        [128, RG * TQB], f32,
                        name=f"pss{b}{j}{g}{hd}", tag=f"ss{hd}",
                    )
                for r in range(g * RG, (g + 1) * RG):
                    for hd in range(NLOC):
                        nc.tensor.matmul(
                            pss[(g % 2, hd)][:, (r % RG) * TQB : (r % RG + 1) * TQB],
                            lhsT=kt[hd * H : (hd + 1) * H, r * TKC : (r + 1) * TKC],
                            rhs=qt[hd * H : (hd + 1) * H, j * TQB : (j + 1) * TQB],
                            start=True,
                            stop=True,
                        )

            def exps(g):
                for hd in range(NLOC):
                    pt[(g % 2, hd)] = p_pt.tile(
                        [128, RG * TQB], bf16,
                        name=f"pt{b}{j}{g}{hd}", tag=f"pt{hd}",
                    )
                    nc.scalar.activation(
                        pt[(g % 2, hd)][:], pss[(g % 2, hd)][:], AF.Exp
                    )

            def pvs(g):
                for hd in range(NLOC):
                    for r in range(g * RG, (g + 1) * RG):
                        col0 = (r * NLOC + hd) * VA
                        nc.tensor.matmul(
                            pv[hd][:],
                            lhsT=va[:, col0 : col0 + VA],
                            rhs=pt[(g % 2, hd)][:, (r % RG) * TQB : (r % RG + 1) * TQB],
                            start=(r == g * RG and g == 0),
                            stop=(r == (g + 1) * RG - 1 and g == NG - 1),
                        )

            scores(0)
            exps(0)
            for g in range(NG):
                if g + 1 < NG:
                    scores(g + 1)
                    exps(g + 1)
                if fill_groups is None or g in fill_groups:
                    run_filler()
                pvs(g)
            # normalize this block inline: its DVE ops enter the DVE queue
            # before the next block's filler evacuations, so staging for the
            # AllToAll fires promptly at block end.
            make_norm(b, j, pv)()

        # ===== main schedule =================================================
        xs0 = load_startup()
        qt0, kt0, va0, pieces0 = make_proj(0, xs0)
        # inline: V, va, K, Q(tb=0); leave Q(tb 1..3) as fillers
        for p in pieces0[:13]:
            p()
        fillers.extend(pieces0[13:])

        xs1 = load_x(1)
        qt1, kt1, va1, pieces1 = make_proj(1, xs1)
        fillers.extend(pieces1)

        for j in range(NTQB):
            attn_block(0, j, qt0, kt0, va0, fill_groups=(0, 2, 4, 6))
        nc.gpsimd.collective_compute(
            "AllToAll",
            ALU.bypass,
            replica_groups=GROUPS,
            ins=[a2a_in[0].opt()],
            outs=[a2a_out[0].opt()],
        )

        # b=1: leftover b=1 Q projections pop in blocks 0-1; the b=0 output
        # projection pieces pop only in block 3, by which time the b=0
        # AllToAll (issued ~90us earlier, overlapped) has completed.
        for j in range(2):
            attn_block(1, j, qt1, kt1, va1, fill_groups=(1, 3))
        fillers.extend(outproj_pieces(0))
        attn_block(1, 2, qt1, kt1, va1, fill_groups=())
        attn_block(1, 3, qt1, kt1, va1, fill_groups=(1, 3, 5, 7))
        nc.gpsimd.collective_compute(
            "AllToAll",
            ALU.bypass,
            replica_groups=GROUPS,
            ins=[a2a_in[1].opt()],
            outs=[a2a_out[1].opt()],
        )
        run_filler(len(fillers))
        for p in outproj_pieces(1):
            p()

    orig_to_json = nc.to_json_bytes
    nc.to_json_bytes = lambda: _legalize_waits(orig_to_json())
    return nc


def _get_nc():
    if "nc" not in _CACHE:
        _CACHE["nc"] = _build()
    return _CACHE["nc"]


def _make_in_maps(inputs):
    q = np.asarray(inputs["q"], dtype=np.float32)
    v = np.asarray(inputs["v"], dtype=np.float32)
    k = np.asarray(inputs["k"], dtype=np.float32)
    w_query = np.asarray(inputs["w_query"], dtype=np.float32)
    b_query = np.asarray(inputs["b_query"], dtype=np.float32)
    w_value = np.asarray(inputs["w_value"], dtype=np.float32)
    b_value = np.asarray(inputs["b_value"], dtype=np.float32)
    w_key = np.asarray(inputs["w_key"], dtype=np.float32)
    b_key = np.asarray(inputs["b_key"], dtype=np.float32)
    w_projection = np.asarray(inputs["w_projection"], dtype=np.float32)
    b_projection = np.asarray(inputs["b_projection"], dtype=np.float32)

    scale = np.float32(1.0 / np.sqrt(H))

    def arrange_w(w):
        # [D or N*H, m] -> SBUF layout [128, (chunk, m)], contiguous rows
        m = w.shape[1]
        return np.ascontiguousarray(
            w.reshape(-1, 128, m).transpose(1, 0, 2).reshape(128, -1)
        ).astype(BF16)

    def arrange_x(xb):
        # [T, D] -> [D, T] -> SBUF layout [128, (tb, dc, t)], contiguous rows
        return np.ascontiguousarray(
            xb.T.reshape(NDC, 128, NTQB, XB)
            .transpose(1, 2, 0, 3)
            .reshape(128, NTQB * NDC * XB)
        ).astype(BF16)

    wp_s = arrange_w(
        np.ascontiguousarray(
            w_projection.transpose(0, 2, 1).reshape(N_HEADS * H, D)
        )
    )
    bp_s = np.ascontiguousarray(
        np.tile(b_projection.reshape(1, D), (128, 1))
    ).astype(np.float32)

    xT = {}
    for b in range(B):
        xT[b] = tuple(arrange_x(x[b]) for x in (q, k, v))

    in_maps = []
    for c in range(NCORES):
        hs = c * NLOC
        wq_s = arrange_w(w_query[:, hs : hs + NLOC, :].reshape(D, NW) * scale)
        wk_s = arrange_w(w_key[:, hs : hs + NLOC, :].reshape(D, NW))
        wv_s = arrange_w(w_value[:, hs : hs + NLOC, :].reshape(D, NW))
        bq_s = np.ascontiguousarray(
            (b_query[hs : hs + NLOC].reshape(NW) * scale).reshape(NW, 1)
        ).astype(np.float32)
        bk_s = np.ascontiguousarray(
            b_key[hs : hs + NLOC].reshape(NW, 1)
        ).astype(np.float32)
        bv_s = np.ascontiguousarray(
            b_value[hs : hs + NLOC].reshape(NW, 1)
        ).astype(np.float32)
        m = {
            "ident": np.eye(128, dtype=np.float32).astype(BF16),
            "identf": np.eye(128, dtype=np.float32),
            "wq": np.ascontiguousarray(wq_s),
            "wk": np.ascontiguousarray(wk_s),
            "wv": np.ascontiguousarray(wv_s),
            "wp": wp_s,
            "bq": bq_s,
            "bk": bk_s,
            "bv": bv_s,
            "bp": bp_s,
        }
        for b in range(B):
            m[f"qT{b}"], m[f"kT{b}"], m[f"vT{b}"] = xT[b]
        in_maps.append(m)
    return in_maps


def _assemble(results):
    out = np.empty((B, T, D), np.float32)
    for c in range(NCORES):
        res = results[c]["out"]
        for b in range(B):
            out[b, c * SLICE : (c + 1) * SLICE, :] = res[
                b * SLICE : (b + 1) * SLICE
            ]
    return out


def run(inputs, trace=False, **kwargs):
    from concourse.bass_utils import run_bass_kernel_spmd

    nc = _get_nc()
    in_maps = _make_in_maps(inputs)
    res = run_bass_kernel_spmd(
        nc, in_maps, list(range(NCORES)), trace=trace, **kwargs
    )
    return _assemble(res.results), res


def kernel(**inputs) -> np.ndarray:
    out, _ = run(inputs, trace=False)
    return out
